# revision 11
# baseline (speedup 1.0000x reference)
"""Masked max-pool over span axis (MaxSpanRepr) on 8 Trainium2 cores.

Computation: out[b, l, d] = max_s( mask[b, s] ? spans[b, l, s, d] : -1e10 )
  spans          [2048, 13, 4, 1024] f32
  attention_mask [2048, 4] int32
  out            [2048, 13, 1024] f32

Strategy: data-parallel over batch, 256 examples per core. Per core the
spans shard is a [13312 x 1024] table of 4KB chunks (chunk index
r*4 + s for row r=(b,l)). Masked chunks are never read: one indirect
gather per 128-row tile pulls only the unmasked chunks (masked slots
carry an out-of-bounds index and are silently skipped), cutting HBM
read traffic roughly in half versus a dense load. The masked max is
then an add-bias/max chain: slot s contributes x + bias where bias is
0 for unmasked and -1e10 for masked slots (skipped slots hold stale
SBUF data with |x| < 512, and x + (-1e10) rounds to exactly -1e10 in
f32, matching the reference's where()). Slot 0 runs on the scalar
engine (activation Identity with per-partition bias), slots 1-3 on the
vector engine as fused (add, max) scalar_tensor_tensor ops. Stores are
dense contiguous 512KB DMAs. Index/bias tables are computed on host
from the 8 KB mask and shipped as small extra inputs, so the NEFF is
input-independent.
"""

import numpy as np

import concourse.bass as bass
import concourse.mybir as mybir
from concourse.bass_utils import run_bass_kernel_spmd
from concourse.tile import TileContext

B, L, S, D = 2048, 13, 4, 1024
N_CORES = 8
B_SH = B // N_CORES              # 256 examples per core
ROWS = B_SH * L                  # 3328 (b,l) rows per core
N_CHUNKS = ROWS * S              # 13312 4KB chunks per core
N_TILES = ROWS // 128            # 26 tiles of 128 rows
NEG_FILL = np.float32(-1e10)
OOB_IDX = np.int32(10 ** 7)      # skip marker: way past bounds_check

_NC_CACHE = {}


# The walrus build in this container supports a single sync-wait slot per
# instruction ("Too many sync wait commands" in setupSyncWait otherwise),
# while Tile freely attaches one wait per semaphore lane. Post-pass: for any
# instruction carrying N>1 waits, hoist N-1 of them onto NoOp instructions
# inserted just before it on the same engine (engines execute in order, so
# all waits still complete before the instruction runs).
def _split_multi_wait_instructions(nc):
    ctr = 0
    for fn in nc.m.functions:
        for blk in fn.blocks:
            insts = blk.instructions
            out = []
            changed = False
            for inst in insts:
                si = inst.sync_info
                waits = list(si.on_wait) if si is not None else []
                if len(waits) > 1:
                    changed = True
                    for w in waits[:-1]:
                        ctr += 1
                        nop = mybir.InstNoOp(
                            name=f"I-waitsplit-{ctr}", ins=[], outs=[])
                        nop.engine = inst.engine
                        nsi = mybir.SyncInfo(on_update=[], on_wait=[w])
                        nop.sync_info = nsi
                        out.append(nop)
                    si.on_wait = [waits[-1]]
                out.append(inst)
            if changed:
                blk.instructions = out


def _build_nc():
    if "nc" in _NC_CACHE:
        return _NC_CACHE["nc"]
    nc = bass.Bass()
    f32, i32 = mybir.dt.float32, mybir.dt.int32
    spans = nc.dram_tensor("spans", [N_CHUNKS, D], f32, kind="ExternalInput")
    idx = nc.dram_tensor("idx", [128, N_TILES * S], i32, kind="ExternalInput")
    bias = nc.dram_tensor("bias", [128, N_TILES * S], f32,
                          kind="ExternalInput")
    out = nc.dram_tensor("out", [ROWS, D], f32, kind="ExternalOutput")

    with TileContext(nc) as tc:
        with (
            tc.tile_pool(name="constp", bufs=1) as const_pool,
            tc.tile_pool(name="inp", bufs=4) as in_pool,
            tc.tile_pool(name="outp", bufs=4) as out_pool,
        ):
            idx_t = const_pool.tile([128, N_TILES * S], i32)
            nc.sync.dma_start(out=idx_t[:], in_=idx[:])
            bounds_reg = nc.gpsimd.to_reg(N_CHUNKS - 1)
            bias_t = const_pool.tile([128, N_TILES * S], f32)
            nc.sync.dma_start(out=bias_t[:], in_=bias[:])

            # Pre-zero the gather buffers once: skipped gather slots leave
            # stale SBUF behind, and the -1e10 bias add is only exact when
            # |stale| < 512. After round one the stale data is old span
            # values (|x| < 6), so zeroing the first use is sufficient.
            for _ in range(4):
                tin = in_pool.tile([128, S * D], f32, tag="tin")
                nc.vector.memset(tin[:], 0.0)

            for t in range(N_TILES):
                tin = in_pool.tile([128, S * D], f32, tag="tin")
                # One indirect gather per slot: the engine consumes one
                # index per partition and moves a dst-extent-sized (4KB)
                # contiguous block; OOB indices skip the partition.
                for m in range(S):
                    nc.gpsimd.indirect_dma_start(
                        out=tin[:, m * D:(m + 1) * D],
                        out_offset=None,
                        in_=spans[:],
                        in_offset=bass.IndirectOffsetOnAxis(
                            ap=idx_t[:, t * S + m:t * S + m + 1], axis=0),
                        bounds_check=bounds_reg,
                        oob_is_err=False,
                    )
                tout = out_pool.tile([128, D], f32, tag="tout")
                nc.scalar.activation(
                    tout[:], tin[:, 0:D],
                    mybir.ActivationFunctionType.Identity,
                    bias=bias_t[:, t * S:t * S + 1],
                )
                for m in range(1, S):
                    nc.vector.scalar_tensor_tensor(
                        out=tout[:], in0=tin[:, m * D:(m + 1) * D],
                        scalar=bias_t[:, t * S + m:t * S + m + 1],
                        in1=tout[:],
                        op0=mybir.AluOpType.add,
                        op1=mybir.AluOpType.max,
                    )
                nc.sync.dma_start(
                    out=out[t * 128:(t + 1) * 128, :], in_=tout[:])

    _split_multi_wait_instructions(nc)
    _NC_CACHE["nc"] = nc
    return nc


def _make_in_maps(spans, attention_mask):
    spans = np.ascontiguousarray(np.asarray(spans, dtype=np.float32))
    mask = np.asarray(attention_mask)
    assert spans.shape == (B, L, S, D), spans.shape
    assert mask.shape == (B, S), mask.shape

    valid = mask != 0                                    # [B, S]
    valid_rows = np.repeat(valid, L, axis=0)             # [B*L, S]
    # The gather hardware compacts valid entries within each partition:
    # the m-th non-skipped chunk lands in slot m regardless of its mask
    # position. Pre-compact the tables to match: slot m = m-th valid s.
    order = np.argsort(~valid_rows, axis=1, kind="stable")   # valid s first
    valid_sorted = np.take_along_axis(valid_rows, order, axis=1)
    chunk = (np.arange(B * L, dtype=np.int32)[:, None] * S
             + np.arange(S, dtype=np.int32)[None, :])    # [B*L, S]
    chunk_sorted = np.take_along_axis(chunk, order, axis=1)
    bias_rows = np.where(valid_sorted, np.float32(0.0), NEG_FILL)
    idx_rows = np.where(valid_sorted, chunk_sorted, OOB_IDX)

    spans_flat = spans.reshape(B * L, S * D)

    in_maps = []
    for i in range(N_CORES):
        sl = slice(i * ROWS, (i + 1) * ROWS)
        # per-core chunk indices are relative to the core's shard
        idx_core = idx_rows[sl] - np.int32(i * ROWS * S)
        idx_core = np.where(valid_sorted[sl], idx_core, OOB_IDX)
        # [3328, S] -> [N_TILES, 128, S] -> [128, N_TILES, S]
        idx_sb = np.ascontiguousarray(
            idx_core.reshape(N_TILES, 128, S).transpose(1, 0, 2)
        ).reshape(128, N_TILES * S)
        bias_sb = np.ascontiguousarray(
            bias_rows[sl].reshape(N_TILES, 128, S).transpose(1, 0, 2)
        ).reshape(128, N_TILES * S)
        in_maps.append({
            "spans": spans_flat[sl].reshape(ROWS * S, D),
            "idx": idx_sb,
            "bias": bias_sb,
        })
    return in_maps


def run(spans, attention_mask, **spmd_kwargs):
    """Run the device kernel; returns (full_output, BassKernelResults)."""
    nc = _build_nc()
    in_maps = _make_in_maps(spans, attention_mask)
    res = run_bass_kernel_spmd(nc, in_maps, core_ids=list(range(N_CORES)),
                               **spmd_kwargs)
    outs = [r["out"] for r in res.results]
    full = np.concatenate(outs, axis=0).reshape(B, L, D)
    return full, res


def kernel(spans, attention_mask):
    full, _ = run(spans, attention_mask)
    return full


# revision 16
# speedup vs baseline: 1.0998x; 1.0998x over previous
"""Masked max-pool over span axis (MaxSpanRepr) on 8 Trainium2 cores.

Computation: out[b, l, d] = max_s( mask[b, s] ? spans[b, l, s, d] : -1e10 )
  spans          [2048, 13, 4, 1024] f32
  attention_mask [2048, 4] int32
  out            [2048, 13, 1024] f32

Strategy: data-parallel over batch, 256 examples per core. Per core the
spans shard is a [13312 x 1024] table of 4KB chunks (chunk index
r*4 + s for row r=(b,l)). Masked chunks are mostly not read. The
indirect-DMA engine consumes one index per partition per instruction
and moves a dst-extent-sized contiguous block (skipping the partition
when the index fails the bounds check), at a fixed ~1.3us issue cost
per instruction - so the kernel uses three indirect gathers per
128-row tile:

  op1 (8KB -> slots 0,1): pair read from the row's first valid chunk
  op2 (4KB -> slot 2):    next uncovered valid chunk, plain write
  op3 (4KB -> slot 2):    last uncovered valid chunk, CCE max-accum
                          in the DMA datapath (any 4-bit mask leaves
                          at most 2 chunks uncovered after the pair)

The masked max is then a 3-slot add-bias/max chain: slot j contributes
x + bias where bias is 0 for wanted chunks and -1e10 for unwanted or
skipped slots (skipped slots hold stale SBUF data with |x| < 512, and
x + (-1e10) rounds to exactly -1e10 in f32, matching the reference's
where()). Slot 0 runs on the scalar engine (activation Identity with
per-partition bias), slots 1-2 on the vector engine as fused (add,max)
scalar_tensor_tensor ops. Stores are dense contiguous 512KB DMAs.
Index/bias tables are computed on host from the 8 KB mask and shipped
as small extra inputs, so the NEFF is input-independent.
"""

import numpy as np

import concourse.bass as bass
import concourse.mybir as mybir
from concourse.bass_utils import run_bass_kernel_spmd
from concourse.tile import TileContext

B, L, S, D = 2048, 13, 4, 1024
N_CORES = 8
B_SH = B // N_CORES              # 256 examples per core
ROWS = B_SH * L                  # 3328 (b,l) rows per core
N_CHUNKS = ROWS * S              # 13312 4KB chunks per core
N_TILES = ROWS // 128            # 26 tiles of 128 rows
N_SLOTS = 4                      # pair (2) + two remainder chunks
N_OPS = 3                        # indirect gathers per tile
NEG_FILL = np.float32(-1e10)
OOB_IDX = np.int32(10 ** 7)      # skip marker: way past bounds_check

_NC_CACHE = {}


# The walrus build in this container supports a single sync-wait slot per
# instruction ("Too many sync wait commands" in setupSyncWait otherwise),
# while Tile freely attaches one wait per semaphore lane. Post-pass: for any
# instruction carrying N>1 waits, hoist N-1 of them onto NoOp instructions
# inserted just before it on the same engine (engines execute in order, so
# all waits still complete before the instruction runs).
def _split_multi_wait_instructions(nc):
    ctr = 0
    for fn in nc.m.functions:
        for blk in fn.blocks:
            insts = blk.instructions
            out = []
            changed = False
            for inst in insts:
                si = inst.sync_info
                waits = list(si.on_wait) if si is not None else []
                if len(waits) > 1:
                    changed = True
                    for w in waits[:-1]:
                        ctr += 1
                        nop = mybir.InstNoOp(
                            name=f"I-waitsplit-{ctr}", ins=[], outs=[])
                        nop.engine = inst.engine
                        nsi = mybir.SyncInfo(on_update=[], on_wait=[w])
                        nop.sync_info = nsi
                        out.append(nop)
                    si.on_wait = [waits[-1]]
                out.append(inst)
            if changed:
                blk.instructions = out


def _build_nc():
    if "nc" in _NC_CACHE:
        return _NC_CACHE["nc"]
    nc = bass.Bass()
    f32, i32 = mybir.dt.float32, mybir.dt.int32
    spans = nc.dram_tensor("spans", [N_CHUNKS, D], f32, kind="ExternalInput")
    idx = nc.dram_tensor("idx", [128, N_TILES * N_OPS], i32,
                         kind="ExternalInput")
    bias = nc.dram_tensor("bias", [128, N_TILES * N_SLOTS], f32,
                          kind="ExternalInput")
    out = nc.dram_tensor("out", [ROWS, D], f32, kind="ExternalOutput")

    with TileContext(nc) as tc:
        with (
            tc.tile_pool(name="constp", bufs=1) as const_pool,
            tc.tile_pool(name="inp", bufs=4) as in_pool,
            tc.tile_pool(name="outp", bufs=4) as out_pool,
        ):
            idx_t = const_pool.tile([128, N_TILES * N_OPS], i32)
            nc.sync.dma_start(out=idx_t[:], in_=idx[:])
            bounds_reg = nc.gpsimd.to_reg(N_CHUNKS - 1)
            bias_t = const_pool.tile([128, N_TILES * N_SLOTS], f32)
            nc.sync.dma_start(out=bias_t[:], in_=bias[:])

            # Pre-zero the gather buffers once: skipped gather slots leave
            # stale SBUF behind, and the -1e10 bias add is only exact when
            # |stale| < 512. After round one the stale data is old span
            # values (|x| < 6), so zeroing the first use is sufficient.
            for _ in range(4):
                tin = in_pool.tile([128, N_SLOTS * D], f32, tag="tin")
                nc.vector.memset(tin[:], 0.0)

            for t in range(N_TILES):
                c = t * N_OPS
                cb = t * N_SLOTS
                tin = in_pool.tile([128, N_SLOTS * D], f32, tag="tin")
                # pair read -> slots 0,1
                nc.gpsimd.indirect_dma_start(
                    out=tin[:, 0:2 * D],
                    out_offset=None,
                    in_=spans[:],
                    in_offset=bass.IndirectOffsetOnAxis(
                        ap=idx_t[:, c:c + 1], axis=0),
                    bounds_check=bounds_reg,
                    oob_is_err=False,
                )
                # remaining (un-covered) chunks -> slots 2 and 3
                nc.gpsimd.indirect_dma_start(
                    out=tin[:, 2 * D:3 * D],
                    out_offset=None,
                    in_=spans[:],
                    in_offset=bass.IndirectOffsetOnAxis(
                        ap=idx_t[:, c + 1:c + 2], axis=0),
                    bounds_check=bounds_reg,
                    oob_is_err=False,
                )
                nc.gpsimd.indirect_dma_start(
                    out=tin[:, 3 * D:4 * D],
                    out_offset=None,
                    in_=spans[:],
                    in_offset=bass.IndirectOffsetOnAxis(
                        ap=idx_t[:, c + 2:c + 3], axis=0),
                    bounds_check=bounds_reg,
                    oob_is_err=False,
                )
                tout = out_pool.tile([128, D], f32, tag="tout")
                nc.scalar.activation(
                    tout[:], tin[:, 0:D],
                    mybir.ActivationFunctionType.Identity,
                    bias=bias_t[:, cb:cb + 1],
                )
                for m in range(1, N_SLOTS):
                    nc.vector.scalar_tensor_tensor(
                        out=tout[:], in0=tin[:, m * D:(m + 1) * D],
                        scalar=bias_t[:, cb + m:cb + m + 1],
                        in1=tout[:],
                        op0=mybir.AluOpType.add,
                        op1=mybir.AluOpType.max,
                    )
                nc.sync.dma_start(
                    out=out[t * 128:(t + 1) * 128, :], in_=tout[:])

    _split_multi_wait_instructions(nc)
    _NC_CACHE["nc"] = nc
    return nc


def _make_tables(valid_rows):
    """Per-row gather plan. valid_rows: [ROWS, S] bool for one core.
    Returns idx [ROWS, 3] int32 (core-local chunk ids or OOB) and
    bias [ROWS, 3] float32."""
    idx = np.full((ROWS, N_OPS), OOB_IDX, np.int32)
    bia = np.full((ROWS, N_SLOTS), NEG_FILL, np.float32)
    for r in range(ROWS):
        vs = np.nonzero(valid_rows[r])[0]
        if len(vs) == 0:
            continue
        v = [r * S + int(s) for s in vs]
        a1 = min(v[0], N_CHUNKS - 2)
        covered = {a1, a1 + 1} & set(v)
        rem = [x for x in v if x not in covered]
        idx[r, 0] = a1
        bia[r, 0] = 0.0 if a1 in covered else NEG_FILL
        bia[r, 1] = 0.0 if (a1 + 1) in covered else NEG_FILL
        if rem:
            idx[r, 1] = rem[0]
            bia[r, 2] = 0.0
        if len(rem) > 1:
            idx[r, 2] = rem[1]
            bia[r, 3] = 0.0
    return idx, bia


def _make_in_maps(spans, attention_mask):
    spans = np.ascontiguousarray(np.asarray(spans, dtype=np.float32))
    mask = np.asarray(attention_mask)
    assert spans.shape == (B, L, S, D), spans.shape
    assert mask.shape == (B, S), mask.shape

    valid = mask != 0                                    # [B, S]
    spans_flat = spans.reshape(B * L, S * D)

    in_maps = []
    for i in range(N_CORES):
        valid_core = np.repeat(valid[i * B_SH:(i + 1) * B_SH], L, axis=0)
        idx_rows, bias_rows = _make_tables(valid_core)
        # bias cols are (slot0, slot1, slot2); idx cols are (op1, op2, op3)
        idx_sb = np.ascontiguousarray(
            idx_rows.reshape(N_TILES, 128, N_OPS).transpose(1, 0, 2)
        ).reshape(128, N_TILES * N_OPS)
        bias_sb = np.ascontiguousarray(
            bias_rows.reshape(N_TILES, 128, N_SLOTS).transpose(1, 0, 2)
        ).reshape(128, N_TILES * N_SLOTS)
        sl = slice(i * ROWS, (i + 1) * ROWS)
        in_maps.append({
            "spans": spans_flat[sl].reshape(ROWS * S, D),
            "idx": idx_sb,
            "bias": bias_sb,
        })
    return in_maps


def run(spans, attention_mask, **spmd_kwargs):
    """Run the device kernel; returns (full_output, BassKernelResults)."""
    nc = _build_nc()
    in_maps = _make_in_maps(spans, attention_mask)
    res = run_bass_kernel_spmd(nc, in_maps, core_ids=list(range(N_CORES)),
                               **spmd_kwargs)
    outs = [r["out"] for r in res.results]
    full = np.concatenate(outs, axis=0).reshape(B, L, D)
    return full, res


def kernel(spans, attention_mask):
    full, _ = run(spans, attention_mask)
    return full


# revision 17
# speedup vs baseline: 1.2068x; 1.0973x over previous
"""Masked max-pool over span axis (MaxSpanRepr) on 8 Trainium2 cores.

Computation: out[b, l, d] = max_s( mask[b, s] ? spans[b, l, s, d] : -1e10 )
  spans          [2048, 13, 4, 1024] f32
  attention_mask [2048, 4] int32
  out            [2048, 13, 1024] f32

Strategy: data-parallel over batch, 256 examples per core. Per core the
spans shard is a [13312 x 1024] table of 4KB chunks (chunk index
r*4 + s for row r=(b,l)). Masked chunks are mostly not read. The
indirect-DMA engine consumes one index per partition per instruction
and moves a dst-extent-sized contiguous block (skipping the partition
when the index fails the bounds check), at a fixed ~1.3us issue cost
per instruction - so the kernel uses three indirect gathers per
128-row tile:

  op1 (8KB -> slots 0,1): pair read from the row's first valid chunk
  op2 (4KB -> slot 2):    next uncovered valid chunk, plain write
  op3 (4KB -> slot 2):    last uncovered valid chunk, CCE max-accum
                          in the DMA datapath (any 4-bit mask leaves
                          at most 2 chunks uncovered after the pair)

The masked max is then a 3-slot add-bias/max chain: slot j contributes
x + bias where bias is 0 for wanted chunks and -1e10 for unwanted or
skipped slots (skipped slots hold stale SBUF data with |x| < 512, and
x + (-1e10) rounds to exactly -1e10 in f32, matching the reference's
where()). Slot 0 runs on the scalar engine (activation Identity with
per-partition bias), slots 1-2 on the vector engine as fused (add,max)
scalar_tensor_tensor ops. Stores are dense contiguous 512KB DMAs.
Index/bias tables are computed on host from the 8 KB mask and shipped
as small extra inputs, so the NEFF is input-independent.
"""

import numpy as np

import concourse.bass as bass
import concourse.mybir as mybir
from concourse.bass_utils import run_bass_kernel_spmd
from concourse.tile import TileContext

B, L, S, D = 2048, 13, 4, 1024
N_CORES = 8
B_SH = B // N_CORES              # 256 examples per core
ROWS = B_SH * L                  # 3328 (b,l) rows per core
N_CHUNKS = ROWS * S              # 13312 4KB chunks per core
N_TILES = ROWS // 128            # 26 tiles of 128 rows
N_SLOTS = 4                      # pair (2) + two remainder chunks
N_OPS = 2                        # indirect gathers per tile
NEG_FILL = np.float32(-1e10)
OOB_IDX = np.int32(10 ** 7)      # skip marker: way past bounds_check

_NC_CACHE = {}


# The walrus build in this container supports a single sync-wait slot per
# instruction ("Too many sync wait commands" in setupSyncWait otherwise),
# while Tile freely attaches one wait per semaphore lane. Post-pass: for any
# instruction carrying N>1 waits, hoist N-1 of them onto NoOp instructions
# inserted just before it on the same engine (engines execute in order, so
# all waits still complete before the instruction runs).
def _split_multi_wait_instructions(nc):
    ctr = 0
    for fn in nc.m.functions:
        for blk in fn.blocks:
            insts = blk.instructions
            out = []
            changed = False
            for inst in insts:
                si = inst.sync_info
                waits = list(si.on_wait) if si is not None else []
                if len(waits) > 1:
                    changed = True
                    for w in waits[:-1]:
                        ctr += 1
                        nop = mybir.InstNoOp(
                            name=f"I-waitsplit-{ctr}", ins=[], outs=[])
                        nop.engine = inst.engine
                        nsi = mybir.SyncInfo(on_update=[], on_wait=[w])
                        nop.sync_info = nsi
                        out.append(nop)
                    si.on_wait = [waits[-1]]
                out.append(inst)
            if changed:
                blk.instructions = out


def _build_nc():
    if "nc" in _NC_CACHE:
        return _NC_CACHE["nc"]
    nc = bass.Bass()
    f32, i32 = mybir.dt.float32, mybir.dt.int32
    spans = nc.dram_tensor("spans", [N_CHUNKS, D], f32, kind="ExternalInput")
    idx = nc.dram_tensor("idx", [128, N_TILES * N_OPS], i32,
                         kind="ExternalInput")
    bias = nc.dram_tensor("bias", [128, N_TILES * N_SLOTS], f32,
                          kind="ExternalInput")
    out = nc.dram_tensor("out", [ROWS, D], f32, kind="ExternalOutput")

    with TileContext(nc) as tc:
        with (
            tc.tile_pool(name="constp", bufs=1) as const_pool,
            tc.tile_pool(name="inp", bufs=6) as in_pool,
            tc.tile_pool(name="outp", bufs=6) as out_pool,
        ):
            idx_t = const_pool.tile([128, N_TILES * N_OPS], i32)
            nc.sync.dma_start(out=idx_t[:], in_=idx[:])
            bounds_reg = nc.gpsimd.to_reg(N_CHUNKS - 1)
            bias_t = const_pool.tile([128, N_TILES * N_SLOTS], f32)
            nc.sync.dma_start(out=bias_t[:], in_=bias[:])

            # Pre-zero the gather buffers once: skipped gather slots leave
            # stale SBUF behind, and the -1e10 bias add is only exact when
            # |stale| < 512. After round one the stale data is old span
            # values (|x| < 6), so zeroing the first use is sufficient.
            for _ in range(6):
                tin = in_pool.tile([128, N_SLOTS * D], f32, tag="tin")
                nc.vector.memset(tin[:], 0.0)

            for t in range(N_TILES):
                c = t * N_OPS
                cb = t * N_SLOTS
                tin = in_pool.tile([128, N_SLOTS * D], f32, tag="tin")
                # pair read -> slots 0,1
                nc.gpsimd.indirect_dma_start(
                    out=tin[:, 0:2 * D],
                    out_offset=None,
                    in_=spans[:],
                    in_offset=bass.IndirectOffsetOnAxis(
                        ap=idx_t[:, c:c + 1], axis=0),
                    bounds_check=bounds_reg,
                    oob_is_err=False,
                )
                # pair read from first uncovered chunk -> slots 2,3
                # (after the first pair, any mask leaves at most 2 wanted
                # chunks and when there are 2 they are adjacent)
                nc.gpsimd.indirect_dma_start(
                    out=tin[:, 2 * D:4 * D],
                    out_offset=None,
                    in_=spans[:],
                    in_offset=bass.IndirectOffsetOnAxis(
                        ap=idx_t[:, c + 1:c + 2], axis=0),
                    bounds_check=bounds_reg,
                    oob_is_err=False,
                )
                tout = out_pool.tile([128, D], f32, tag="tout")
                nc.scalar.activation(
                    tout[:], tin[:, 0:D],
                    mybir.ActivationFunctionType.Identity,
                    bias=bias_t[:, cb:cb + 1],
                )
                for m in range(1, N_SLOTS):
                    nc.vector.scalar_tensor_tensor(
                        out=tout[:], in0=tin[:, m * D:(m + 1) * D],
                        scalar=bias_t[:, cb + m:cb + m + 1],
                        in1=tout[:],
                        op0=mybir.AluOpType.add,
                        op1=mybir.AluOpType.max,
                    )
                nc.sync.dma_start(
                    out=out[t * 128:(t + 1) * 128, :], in_=tout[:])

    _split_multi_wait_instructions(nc)
    _NC_CACHE["nc"] = nc
    return nc


def _make_tables(valid_rows):
    """Per-row gather plan. valid_rows: [ROWS, S] bool for one core.
    Returns idx [ROWS, 3] int32 (core-local chunk ids or OOB) and
    bias [ROWS, 3] float32."""
    idx = np.full((ROWS, N_OPS), OOB_IDX, np.int32)
    bia = np.full((ROWS, N_SLOTS), NEG_FILL, np.float32)
    for r in range(ROWS):
        vs = np.nonzero(valid_rows[r])[0]
        if len(vs) == 0:
            continue
        v = [r * S + int(s) for s in vs]
        a1 = min(v[0], N_CHUNKS - 2)
        covered = {a1, a1 + 1} & set(v)
        rem = [x for x in v if x not in covered]
        idx[r, 0] = a1
        bia[r, 0] = 0.0 if a1 in covered else NEG_FILL
        bia[r, 1] = 0.0 if (a1 + 1) in covered else NEG_FILL
        if rem:
            a2 = min(rem[0], N_CHUNKS - 2)
            idx[r, 1] = a2
            remset = set(rem)
            bia[r, 2] = 0.0 if a2 in remset else NEG_FILL
            bia[r, 3] = 0.0 if (a2 + 1) in remset else NEG_FILL
    return idx, bia


def _make_in_maps(spans, attention_mask):
    spans = np.ascontiguousarray(np.asarray(spans, dtype=np.float32))
    mask = np.asarray(attention_mask)
    assert spans.shape == (B, L, S, D), spans.shape
    assert mask.shape == (B, S), mask.shape

    valid = mask != 0                                    # [B, S]
    spans_flat = spans.reshape(B * L, S * D)

    in_maps = []
    for i in range(N_CORES):
        valid_core = np.repeat(valid[i * B_SH:(i + 1) * B_SH], L, axis=0)
        idx_rows, bias_rows = _make_tables(valid_core)
        # bias cols are (slot0, slot1, slot2); idx cols are (op1, op2, op3)
        idx_sb = np.ascontiguousarray(
            idx_rows.reshape(N_TILES, 128, N_OPS).transpose(1, 0, 2)
        ).reshape(128, N_TILES * N_OPS)
        bias_sb = np.ascontiguousarray(
            bias_rows.reshape(N_TILES, 128, N_SLOTS).transpose(1, 0, 2)
        ).reshape(128, N_TILES * N_SLOTS)
        sl = slice(i * ROWS, (i + 1) * ROWS)
        in_maps.append({
            "spans": spans_flat[sl].reshape(ROWS * S, D),
            "idx": idx_sb,
            "bias": bias_sb,
        })
    return in_maps


def run(spans, attention_mask, **spmd_kwargs):
    """Run the device kernel; returns (full_output, BassKernelResults)."""
    nc = _build_nc()
    in_maps = _make_in_maps(spans, attention_mask)
    res = run_bass_kernel_spmd(nc, in_maps, core_ids=list(range(N_CORES)),
                               **spmd_kwargs)
    outs = [r["out"] for r in res.results]
    full = np.concatenate(outs, axis=0).reshape(B, L, D)
    return full, res


def kernel(spans, attention_mask):
    full, _ = run(spans, attention_mask)
    return full
